# revision 10
# baseline (speedup 1.0000x reference)
"""Trainium2 Bass kernel for CustomGATConv (dense masked GAT attention).

Strategy (8-core SPMD, row-sharded attention):
  - Each core owns 512 destination rows i of the [4096, 4096, 8] attention
    tensor.  Inputs are node-rotated per core so that the identical program
    always works on rows [0:512) of its own rotated node order.
  - h = x @ W is computed on every core (replicated, cheap on PE).
  - Per (row-block, head): z[j, i] = e_src[i] + e_dst[j] + (-200 if masked)
    is built entirely in PSUM by three tiny matmuls (rank-1/2 outer products
    plus an identity-weighted mask inject), so the ScalarEngine only runs
    two activation passes: Prelu(alpha=0.2) then Exp.  exp(-200ish) == 0
    implements the mask.
  - alpha @ h and the softmax denominator come from one accumulated matmul
    against h augmented with a ones column ([K=128 j, 65]).
  - Normalization: PE-transpose of the [65, 512] accumulator, then a DVE
    reciprocal + per-partition scalar multiply.
"""

import re

import numpy as np
import ml_dtypes

import bass_rust as br
import concourse.bass as bass
import concourse.tile as tile
from concourse import mybir
from concourse.bass_utils import run_bass_kernel_spmd

N = 4096
IN = 256
H = 8
F = 64
NCORES = 8
R = N // NCORES          # 512 destination rows per core
JT = N // 128            # 32 j-tiles
KC = IN // 128           # 2 contraction chunks for x @ W
NEG = -200.0             # additive mask value
FP = mybir.dt.float32
BF = mybir.dt.bfloat16


class _TileContext(tile.TileContext):
    """TileContext whose final drain splits its semaphore waits one per
    instruction — this walrus's CTRL_NO encoding only fits one sync wait."""

    def _drain_and_barrier(self, tick_clock, wait_clock):
        gc = tick_clock.global_clock
        vals = list(map(int, re.findall(r"\d+", repr(gc))))
        nonzero = [(i, t) for i, t in enumerate(vals) if t > 0]
        prev = br.VectorClock()
        partial = br.VectorClock()
        for i, t in nonzero:
            partial.require_at_least(i, t)
            inst = self.nc.sync.drain().ins
            wait_clock.add_sem_waits(
                inst,
                br.ScopedClock({None: partial.copy()}),
                br.ScopedClock({None: prev.copy()}),
            )
            prev = partial.copy()
        drain_inst = self.nc.sync.drain().ins
        wait_clock.add_sem_waits(
            drain_inst,
            br.ScopedClock({None: gc}),
            br.ScopedClock({None: prev.copy()}),
        )
        self.nc.all_engine_barrier()
        popped = self.nc._tile_sem_poison_stack.pop()
        assert popped is self._sem_poison
        self.nc.clear_and_free_semaphores(list(self.sems.allocated().values()))
        self.nc.all_engine_barrier()


def _split_excess_waits(nc, cap_compute=1, cap_nop=1):
    """This walrus encodes at most ~2 sync waits per compute instruction and
    1 per CTRL_NO (nop/drain).  Move excess waits onto injected same-engine
    nops placed immediately before the over-subscribed instruction."""
    n_split = 0
    for fn in nc.m.functions:
        for bb in fn.blocks:
            lst = bb.instructions
            i = 0
            while i < len(lst):
                inst = lst[i]
                si = inst.sync_info
                waits = list(si.on_wait) if si is not None else []
                is_ctrl = isinstance(inst, (mybir.InstNoOp, mybir.InstDrain))
                cap = cap_nop if is_ctrl else cap_compute
                if len(waits) > cap:
                    excess, keep = waits[:-cap], waits[-cap:]
                    for w in excess:
                        nop = mybir.InstNoOp(name=f"waitsplit-{nc.next_id()}")
                        nop.engine = inst.engine
                        nop.sync_info = br.SyncInfo(on_wait=[w], on_update=[])
                        lst.insert(i, nop)
                        i += 1
                        n_split += 1
                    inst.sync_info = br.SyncInfo(
                        on_wait=keep, on_update=list(si.on_update)
                    )
                i += 1
    return n_split


def _build_program(repeat=1):
    nc = bass.Bass("TRN2", target_bir_lowering=False, debug=False)
    ap = {}
    ap["xT"] = nc.dram_tensor("xT", [IN, N], FP, kind="ExternalInput").ap()
    ap["w"] = nc.dram_tensor("w", [IN, H * F], FP, kind="ExternalInput").ap()
    ap["wa"] = nc.dram_tensor("wa", [IN, 2 * H], FP, kind="ExternalInput").ap()
    ap["maskadd"] = nc.dram_tensor("maskadd", [N, R], BF, kind="ExternalInput").ap()
    ap["identb"] = nc.dram_tensor("identb", [128, 128], BF, kind="ExternalInput").ap()
    ap["identf"] = nc.dram_tensor("identf", [128, 128], FP, kind="ExternalInput").ap()
    out_ap = nc.dram_tensor("out", [R, H * F], FP, kind="ExternalOutput").ap()

    with _TileContext(nc) as tc:
        _emit(tc, nc, ap, out_ap, repeat)
    _split_excess_waits(nc)
    return nc


def _emit(tc, nc, ap, out_ap, repeat):
    from contextlib import ExitStack

    Act = mybir.ActivationFunctionType
    with ExitStack() as ctx:
        singles = ctx.enter_context(tc.tile_pool(name="singles", bufs=1))

        # ---- persistent tiles ----
        mask_sb = singles.tile([128, JT, R], BF)
        nc.sync.dma_start(mask_sb[:], ap["maskadd"].rearrange("(jt p) i -> p jt i", p=128))
        identb_sb = singles.tile([128, 128], BF)
        nc.sync.dma_start(identb_sb[:], ap["identb"])
        identf_sb = singles.tile([128, 128], FP)
        nc.sync.dma_start(identf_sb[:], ap["identf"])

        haug_sb = singles.tile([128, JT, H, F + 1], FP)
        nc.vector.memset(haug_sb[:, :, :, F:F + 1], 1.0)
        esd_sb = singles.tile([16, N], FP)
        esrc0 = singles.tile([1, H, R], FP)   # e_src rows staged at partition 0
        ones_sb = singles.tile([1, R], FP)
        nc.vector.memset(ones_sb[:], 1.0)
        outsb = singles.tile([128, 4, H * F], FP)

        # ---- stage B: h = x @ W (node-major), esdT = (x @ WA)^T ----
        with tc.tile_pool(name="bigin", bufs=1) as bigin, \
             tc.tile_pool(name="hpsum", bufs=2, space="PSUM") as hpsum:
            xT_sb = bigin.tile([128, KC, N], FP)
            nc.sync.dma_start(xT_sb[:], ap["xT"].rearrange("(k p) n -> p k n", p=128))
            w_sb = bigin.tile([128, KC, H * F], FP)
            nc.sync.dma_start(w_sb[:], ap["w"].rearrange("(k p) f -> p k f", p=128))
            wa_sb = bigin.tile([128, KC, 2 * H], FP)
            nc.sync.dma_start(wa_sb[:], ap["wa"].rearrange("(k p) f -> p k f", p=128))

            for m in range(JT):
                ph = hpsum.tile([128, H * F], FP, tag="ph")
                for k in range(KC):
                    nc.tensor.matmul(
                        ph[:],
                        lhsT=xT_sb[:, k, m * 128:(m + 1) * 128],
                        rhs=w_sb[:, k, :],
                        start=(k == 0),
                        stop=(k == KC - 1),
                    )
                nc.vector.tensor_copy(
                    out=haug_sb[:, m, :, 0:F],
                    in_=ph[:].rearrange("p (h f) -> p h f", h=H),
                )
            for q in range(8):
                pe = hpsum.tile([16, R], FP, tag="pe")
                for k in range(KC):
                    nc.tensor.matmul(
                        pe[:],
                        lhsT=wa_sb[:, k, :],
                        rhs=xT_sb[:, k, q * R:(q + 1) * R],
                        start=(k == 0),
                        stop=(k == KC - 1),
                    )
                nc.vector.tensor_copy(out=esd_sb[:, q * R:(q + 1) * R], in_=pe[:])

        # stage e_src rows (partitions 0..7) down to partition 0 for PE rhs
        # use.  Compute engines can only address partition bases {0,32,64};
        # DMA has no such restriction, so stage via SBUF->SBUF DMA.
        for h in range(H):
            nc.gpsimd.dma_start(out=esrc0[0:1, h, :], in_=esd_sb[h:h + 1, 0:R])

        # ---- stage C: masked softmax + alpha @ h, two heads per pass ----
        dstage = ctx.enter_context(tc.tile_pool(name="dstage", bufs=8))
        zpool = ctx.enter_context(tc.tile_pool(name="zpool", bufs=2, space="PSUM"))
        opool = ctx.enter_context(tc.tile_pool(name="opool", bufs=1, space="PSUM"))
        tpool = ctx.enter_context(tc.tile_pool(name="tpool", bufs=2, space="PSUM"))
        lpool = ctx.enter_context(tc.tile_pool(name="lpool", bufs=3))
        ppool = ctx.enter_context(tc.tile_pool(name="ppool", bufs=3))
        npool = ctx.enter_context(tc.tile_pool(name="npool", bufs=2))

        for _rep in range(repeat):
            for hg in range(H // 2):
                h0 = 2 * hg
                pout = opool.tile([F + 1, 2 * R], FP, tag="pout")
                for jt in range(JT):
                    pz = zpool.tile([128, 2 * R], FP, tag="pz")
                    for hl in range(2):
                        h = h0 + hl
                        sl = slice(hl * R, (hl + 1) * R)
                        # z = e_src[i] (rank-1 broadcast over rows j)
                        nc.tensor.matmul(
                            pz[:, sl],
                            lhsT=ones_sb[:, 0:128],
                            rhs=esrc0[:, h, :],
                            start=True,
                            stop=False,
                            skip_group_check=True,
                        )
                        # z += e_dst[j] (rank-1 broadcast over cols i);
                        # stage the e_dst row slice down to partition 0 first
                        dst_row = dstage.tile([1, 128], FP, tag="dst_row")
                        nc.gpsimd.dma_start(
                            out=dst_row[:],
                            in_=esd_sb[8 + h:9 + h, jt * 128:(jt + 1) * 128],
                        )
                        nc.tensor.matmul(
                            pz[:, sl],
                            lhsT=dst_row[:],
                            rhs=ones_sb[:],
                            start=False,
                            stop=False,
                            skip_group_check=True,
                        )
                        # z += maskadd[j, i]  (identity-weighted inject)
                        nc.tensor.matmul(
                            pz[:, sl],
                            lhsT=identb_sb[:],
                            rhs=mask_sb[:, jt, :],
                            start=False,
                            stop=True,
                            skip_group_check=True,
                        )
                    zl = lpool.tile([128, 2 * R], FP, tag="zl")
                    nc.scalar.activation(out=zl[:], in_=pz[:], func=Act.Prelu, alpha=0.2)
                    pp = ppool.tile([128, 2 * R], FP, tag="pp")
                    nc.scalar.activation(out=pp[:], in_=zl[:], func=Act.Exp)
                    for hl in range(2):
                        sl = slice(hl * R, (hl + 1) * R)
                        nc.tensor.matmul(
                            pout[:, sl],
                            lhsT=haug_sb[:, jt, h0 + hl, :],
                            rhs=pp[:, sl],
                            start=(jt == 0),
                            stop=(jt == JT - 1),
                            skip_group_check=True,
                        )
                # normalize: out[i, f] = pout[f, i] / pout[F, i]
                osb = npool.tile([F + 1, 2 * R], FP, tag="osb")
                nc.vector.tensor_copy(out=osb[:], in_=pout[:])
                for hl in range(2):
                    for ic in range(4):
                        pt = tpool.tile([128, F + 1], FP, tag="pt")
                        nc.tensor.transpose(
                            pt[:],
                            osb[:, hl * R + ic * 128:hl * R + (ic + 1) * 128],
                            identf_sb[0:F + 1, 0:F + 1],
                        )
                        rc = npool.tile([128, 1], FP, tag="rc")
                        nc.vector.reciprocal(rc[:], pt[:, F:F + 1])
                        nc.vector.tensor_scalar_mul(
                            outsb[:, ic, (h0 + hl) * F:(h0 + hl + 1) * F],
                            pt[:, 0:F],
                            rc[:],
                        )

        nc.sync.dma_start(
            out_ap.rearrange("(ic p) f -> p ic f", p=128),
            outsb[:],
        )


def _host_prep(x, edge_index, W, a):
    x = np.asarray(x, np.float32)
    W = np.asarray(W, np.float32)
    a = np.asarray(a, np.float32)
    src = np.asarray(edge_index[0]).astype(np.int64)
    dst = np.asarray(edge_index[1]).astype(np.int64)

    A = np.zeros((H * F, 2 * H), np.float32)
    for h in range(H):
        A[h * F:(h + 1) * F, h] = a[h, :F]
        A[h * F:(h + 1) * F, 8 + h] = a[h, F:]
    wa = np.ascontiguousarray(W @ A)

    maskadd = np.full((NCORES, N, R), NEG, np.float32)
    c_of = src // R
    i_loc = src % R
    r = (dst - c_of * R) % N
    maskadd[c_of, r, i_loc] = 0.0
    idx = np.arange(R)
    maskadd[:, idx, idx] = 0.0
    maskadd = maskadd.astype(ml_dtypes.bfloat16)

    identb = np.eye(128, dtype=ml_dtypes.bfloat16)
    identf = np.eye(128, dtype=np.float32)

    in_maps = []
    for c in range(NCORES):
        xT_c = np.ascontiguousarray(np.roll(x, -c * R, axis=0).T)
        in_maps.append({
            "xT": xT_c,
            "w": W,
            "wa": wa,
            "maskadd": np.ascontiguousarray(maskadd[c]),
            "identb": identb,
            "identf": identf,
        })
    return in_maps


_CACHED = {}


def _get_program(repeat=1):
    if repeat not in _CACHED:
        _CACHED[repeat] = _build_program(repeat)
    return _CACHED[repeat]


def kernel(x, edge_index, W, a, _repeat=1):
    nc = _get_program(_repeat)
    in_maps = _host_prep(x, edge_index, W, a)
    res = run_bass_kernel_spmd(nc, in_maps, core_ids=list(range(NCORES)))
    out = np.concatenate([res.results[c]["out"] for c in range(NCORES)], axis=0)
    return out.astype(np.float32)


# revision 30
# speedup vs baseline: 1.4516x; 1.4516x over previous
"""Trainium2 Bass kernel for CustomGATConv (dense masked GAT attention).

Strategy (8-core SPMD, row-sharded attention):
  - Each core owns 512 destination rows i of the [4096, 4096, 8] attention
    tensor.  Inputs are node-rotated per core so that the identical program
    always works on rows [0:512) of its own rotated node order.
  - h = x @ W is computed on every core (replicated, cheap on PE).
  - Per (row-block, head): z[j, i] = e_src[i] + e_dst[j] + (-200 if masked)
    is built entirely in PSUM by three tiny matmuls (rank-1/2 outer products
    plus an identity-weighted mask inject), so the ScalarEngine only runs
    two activation passes: Prelu(alpha=0.2) then Exp.  exp(-200ish) == 0
    implements the mask.
  - alpha @ h and the softmax denominator come from one accumulated matmul
    against h augmented with a ones column ([K=128 j, 65]).
  - Normalization: PE-transpose of the [65, 512] accumulator, then a DVE
    reciprocal + per-partition scalar multiply.
"""

import re

import numpy as np
import ml_dtypes

import bass_rust as br
import concourse.bass as bass
import concourse.tile as tile
from concourse import mybir
from concourse.bass_utils import run_bass_kernel_spmd

N = 4096
IN = 256
H = 8
F = 64
NCORES = 8
R = N // NCORES          # 512 destination rows per core
JT = N // 128            # 32 j-tiles
KC = IN // 128           # 2 contraction chunks for x @ W
NEG = -200.0             # additive mask value
FP = mybir.dt.float32
BF = mybir.dt.bfloat16
F16 = mybir.dt.float16


class _TileContext(tile.TileContext):
    """TileContext whose final drain splits its semaphore waits one per
    instruction — this walrus's CTRL_NO encoding only fits one sync wait."""

    def _drain_and_barrier(self, tick_clock, wait_clock):
        gc = tick_clock.global_clock
        vals = list(map(int, re.findall(r"\d+", repr(gc))))
        nonzero = [(i, t) for i, t in enumerate(vals) if t > 0]
        prev = br.VectorClock()
        partial = br.VectorClock()
        for i, t in nonzero:
            partial.require_at_least(i, t)
            inst = self.nc.sync.drain().ins
            wait_clock.add_sem_waits(
                inst,
                br.ScopedClock({None: partial.copy()}),
                br.ScopedClock({None: prev.copy()}),
            )
            prev = partial.copy()
        drain_inst = self.nc.sync.drain().ins
        wait_clock.add_sem_waits(
            drain_inst,
            br.ScopedClock({None: gc}),
            br.ScopedClock({None: prev.copy()}),
        )
        self.nc.all_engine_barrier()
        popped = self.nc._tile_sem_poison_stack.pop()
        assert popped is self._sem_poison
        self.nc.clear_and_free_semaphores(list(self.sems.allocated().values()))
        self.nc.all_engine_barrier()


def _split_excess_waits(nc, cap_compute=1, cap_nop=1):
    """This walrus encodes at most ~2 sync waits per compute instruction and
    1 per CTRL_NO (nop/drain).  Move excess waits onto injected same-engine
    nops placed immediately before the over-subscribed instruction."""
    n_split = 0
    for fn in nc.m.functions:
        for bb in fn.blocks:
            lst = bb.instructions
            i = 0
            while i < len(lst):
                inst = lst[i]
                si = inst.sync_info
                waits = list(si.on_wait) if si is not None else []
                is_ctrl = isinstance(inst, (mybir.InstNoOp, mybir.InstDrain))
                cap = cap_nop if is_ctrl else cap_compute
                if len(waits) > cap:
                    excess, keep = waits[:-cap], waits[-cap:]
                    for w in excess:
                        nop = mybir.InstNoOp(name=f"waitsplit-{nc.next_id()}")
                        nop.engine = inst.engine
                        nop.sync_info = br.SyncInfo(on_wait=[w], on_update=[])
                        lst.insert(i, nop)
                        i += 1
                        n_split += 1
                    inst.sync_info = br.SyncInfo(
                        on_wait=keep, on_update=list(si.on_update)
                    )
                i += 1
    return n_split


def _build_program(repeat=1):
    nc = bass.Bass("TRN2", target_bir_lowering=False, debug=False)
    ap = {}
    ap["xT"] = nc.dram_tensor("xT", [IN, N], FP, kind="ExternalInput").ap()
    ap["w"] = nc.dram_tensor("w", [IN, H * F], FP, kind="ExternalInput").ap()
    ap["wa"] = nc.dram_tensor("wa", [IN, 2 * H], FP, kind="ExternalInput").ap()
    ap["maskadd"] = nc.dram_tensor("maskadd", [N, R], BF, kind="ExternalInput").ap()
    ap["identb"] = nc.dram_tensor("identb", [128, 128], BF, kind="ExternalInput").ap()
    ap["identf"] = nc.dram_tensor("identf", [128, 128], FP, kind="ExternalInput").ap()
    ap["onesh"] = nc.dram_tensor("onesh", [1, H, R], FP, kind="ExternalInput").ap()
    out_ap = nc.dram_tensor("out", [R, H * F], FP, kind="ExternalOutput").ap()

    with _TileContext(nc) as tc:
        _emit(tc, nc, ap, out_ap, repeat)
    _split_excess_waits(nc)
    return nc


def _emit(tc, nc, ap, out_ap, repeat):
    from contextlib import ExitStack

    Act = mybir.ActivationFunctionType
    with ExitStack() as ctx:
        singles = ctx.enter_context(tc.tile_pool(name="singles", bufs=1))

        # ---- persistent tiles ----
        mask_sb = singles.tile([128, JT, R], BF)
        nc.sync.dma_start(mask_sb[:], ap["maskadd"].rearrange("(jt p) i -> p jt i", p=128))
        identb_sb = singles.tile([128, 128], BF)
        nc.sync.dma_start(identb_sb[:], ap["identb"])
        identf_sb = singles.tile([128, 128], FP)
        nc.sync.dma_start(identf_sb[:], ap["identf"])

        haug_sb = singles.tile([128, JT, H, F + 1], FP)
        nc.vector.memset(haug_sb[:, :, :, F:F + 1], 1.0)
        esd_sb = singles.tile([16, N], FP)
        # zsrc2[{0,32}, h, :] = e_src row of head h; zsrc2[{1,33}, h, :] =
        # ones — the K=2 z-matmul rhs, replicated at partition bases 0 and
        # 32 because lhsT and rhs must share their base partition.
        zsrc2 = singles.tile([34, H, R], FP)
        nc.sync.dma_start(out=zsrc2[1:2, :, :], in_=ap["onesh"])
        nc.sync.dma_start(out=zsrc2[33:34, :, :], in_=ap["onesh"])
        # dst_quad[:, s, :]: lhsT pairs (manual ping-pong on s).  Rows 0/32
        # are all-ones; rows 1/33 receive the two e_dst row slices by DMA
        # each iteration (PE lhsT base partition must be 0/32/64).
        dst_quad = singles.tile([34, 2, 128], FP)
        nc.vector.memset(dst_quad[0:1, :, :], 1.0)
        nc.vector.memset(dst_quad[32:33, :, :], 1.0)
        outsb = singles.tile([128, 4, H * F], FP)

        # ---- stage B: h = x @ W (node-major), esdT = (x @ WA)^T ----
        with tc.tile_pool(name="bigin", bufs=1) as bigin, \
             tc.tile_pool(name="hpsum", bufs=2, space="PSUM") as hpsum:
            xT_sb = bigin.tile([128, KC, N], FP)
            nc.sync.dma_start(xT_sb[:], ap["xT"].rearrange("(k p) n -> p k n", p=128))
            w_sb = bigin.tile([128, KC, H * F], FP)
            nc.sync.dma_start(w_sb[:], ap["w"].rearrange("(k p) f -> p k f", p=128))
            wa_sb = bigin.tile([128, KC, 2 * H], FP)
            nc.sync.dma_start(wa_sb[:], ap["wa"].rearrange("(k p) f -> p k f", p=128))

            for m in range(JT):
                ph = hpsum.tile([128, H * F], FP, tag="ph")
                for k in range(KC):
                    nc.tensor.matmul(
                        ph[:],
                        lhsT=xT_sb[:, k, m * 128:(m + 1) * 128],
                        rhs=w_sb[:, k, :],
                        start=(k == 0),
                        stop=(k == KC - 1),
                    )
                nc.vector.tensor_copy(
                    out=haug_sb[:, m, :, 0:F],
                    in_=ph[:].rearrange("p (h f) -> p h f", h=H),
                )
            for q in range(8):
                pe = hpsum.tile([16, R], FP, tag="pe")
                for k in range(KC):
                    nc.tensor.matmul(
                        pe[:],
                        lhsT=wa_sb[:, k, :],
                        rhs=xT_sb[:, k, q * R:(q + 1) * R],
                        start=(k == 0),
                        stop=(k == KC - 1),
                    )
                nc.vector.tensor_copy(out=esd_sb[:, q * R:(q + 1) * R], in_=pe[:])

        # stage all e_src rows (partitions 0..7) into row 0 of zsrc2 in one
        # SBUF->SBUF DMA.  Compute engines can only address partition bases
        # {0,32,64}; DMA has no such restriction.
        nc.gpsimd.dma_start(
            out=zsrc2[0:1, :, :],
            in_=esd_sb[0:8, 0:R],
        )
        nc.gpsimd.dma_start(
            out=zsrc2[32:33, :, :],
            in_=esd_sb[0:8, 0:R],
        )

        # ---- stage C: masked softmax + alpha @ h, two heads per pass ----
        zpool = ctx.enter_context(tc.tile_pool(name="zpool", bufs=2, space="PSUM"))
        opool = ctx.enter_context(tc.tile_pool(name="opool", bufs=1, space="PSUM"))
        tpool = ctx.enter_context(tc.tile_pool(name="tpool", bufs=2, space="PSUM"))
        lpool = ctx.enter_context(tc.tile_pool(name="lpool", bufs=3))
        ppool = ctx.enter_context(tc.tile_pool(name="ppool", bufs=3))
        npool = ctx.enter_context(tc.tile_pool(name="npool", bufs=2))

        for _rep in range(repeat):
            for hg in range(H // 2):
                h0 = 2 * hg
                pout = opool.tile([F + 1, 2 * R], FP, tag="pout")
                for jt in range(JT):
                    pz = zpool.tile([128, 2 * R], FP, tag="pz")
                    s = jt % 2
                    # stage this tile's two e_dst row slices to partitions 1/33
                    nc.gpsimd.dma_start(
                        out=dst_quad[1:34:32, s, :],
                        in_=esd_sb[8 + h0:10 + h0, jt * 128:(jt + 1) * 128],
                    )
                    # z[j, i] = e_src[i] + e_dst[j], one K=2 matmul per head
                    for hl in range(2):
                        b = 32 * hl
                        nc.tensor.matmul(
                            pz[:, hl * R:(hl + 1) * R],
                            lhsT=dst_quad[b:b + 2, s, :],
                            rhs=zsrc2[b:b + 2, h0 + hl, :],
                            start=True,
                            stop=False,
                            skip_group_check=True,
                        )
                    # z += maskadd[j, i]: identity-weighted inject, once per
                    # free-dim half (moving operand is capped at 512 here)
                    for hl in range(2):
                        nc.tensor.matmul(
                            pz[:, hl * R:(hl + 1) * R],
                            lhsT=identb_sb[:],
                            rhs=mask_sb[:, jt, :],
                            start=False,
                            stop=True,
                            skip_group_check=True,
                        )
                    zl = lpool.tile([128, 2 * R], FP, tag="zl")
                    nc.scalar.activation(out=zl[:], in_=pz[:], func=Act.Prelu, alpha=0.2)
                    pp = ppool.tile([128, 2 * R], FP, tag="pp")
                    nc.scalar.activation(out=pp[:], in_=zl[:], func=Act.Exp)
                    for hl in range(2):
                        sl = slice(hl * R, (hl + 1) * R)
                        nc.tensor.matmul(
                            pout[:, sl],
                            lhsT=haug_sb[:, jt, h0 + hl, :],
                            rhs=pp[:, sl],
                            start=(jt == 0),
                            stop=(jt == JT - 1),
                            skip_group_check=True,
                        )
                # normalize: out[i, f] = pout[f, i] / pout[F, i]
                osb = npool.tile([F + 1, 2 * R], FP, tag="osb")
                nc.vector.tensor_copy(out=osb[:], in_=pout[:])
                for hl in range(2):
                    for ic in range(4):
                        pt = tpool.tile([128, F + 1], FP, tag="pt")
                        nc.tensor.transpose(
                            pt[:],
                            osb[:, hl * R + ic * 128:hl * R + (ic + 1) * 128],
                            identf_sb[0:F + 1, 0:F + 1],
                        )
                        rc = npool.tile([128, 1], FP, tag="rc")
                        nc.vector.reciprocal(rc[:], pt[:, F:F + 1])
                        nc.vector.tensor_scalar_mul(
                            outsb[:, ic, (h0 + hl) * F:(h0 + hl + 1) * F],
                            pt[:, 0:F],
                            rc[:],
                        )

        nc.sync.dma_start(
            out_ap.rearrange("(ic p) f -> p ic f", p=128),
            outsb[:],
        )


def _host_prep(x, edge_index, W, a):
    x = np.asarray(x, np.float32)
    W = np.asarray(W, np.float32)
    a = np.asarray(a, np.float32)
    src = np.asarray(edge_index[0]).astype(np.int64)
    dst = np.asarray(edge_index[1]).astype(np.int64)

    A = np.zeros((H * F, 2 * H), np.float32)
    for h in range(H):
        A[h * F:(h + 1) * F, h] = a[h, :F]
        A[h * F:(h + 1) * F, 8 + h] = a[h, F:]
    wa = np.ascontiguousarray(W @ A)

    maskadd = np.full((NCORES, N, R), NEG, np.float32)
    c_of = src // R
    i_loc = src % R
    r = (dst - c_of * R) % N
    maskadd[c_of, r, i_loc] = 0.0
    idx = np.arange(R)
    maskadd[:, idx, idx] = 0.0
    maskadd = maskadd.astype(ml_dtypes.bfloat16)

    identb = np.eye(128, dtype=ml_dtypes.bfloat16)
    identf = np.eye(128, dtype=np.float32)
    onesh = np.ones((1, H, R), np.float32)

    in_maps = []
    for c in range(NCORES):
        xT_c = np.ascontiguousarray(np.roll(x, -c * R, axis=0).T)
        in_maps.append({
            "xT": xT_c,
            "w": W,
            "wa": wa,
            "maskadd": np.ascontiguousarray(maskadd[c]),
            "identb": identb,
            "identf": identf,
            "onesh": onesh,
        })
    return in_maps


_CACHED = {}


def _get_program(repeat=1):
    if repeat not in _CACHED:
        _CACHED[repeat] = _build_program(repeat)
    return _CACHED[repeat]


def kernel(x, edge_index, W, a, _repeat=1):
    nc = _get_program(_repeat)
    in_maps = _host_prep(x, edge_index, W, a)
    res = run_bass_kernel_spmd(nc, in_maps, core_ids=list(range(NCORES)))
    out = np.concatenate([res.results[c]["out"] for c in range(NCORES)], axis=0)
    return out.astype(np.float32)
